# revision 13
# baseline (speedup 1.0000x reference)
"""Trainium2 Bass kernel for the DualLoss nn.Module (v3).

dist[b,m,s,n] = ||P[b,m,s] - X[b,n,m]||^2 via bf16 hi/lo-split matmuls
(K=15 contraction rows per m), with 4x PE row-tiling so four matmuls run
concurrently in the 128x128 array (tile_position=(32j,0)).

  Phase A (d2, per (b,mquad)): 4 banks [s=128, n-chunk 512], one m per
    row group (K=15). Even chunks staged PSUM->SBUF by the scalar engine;
    odd chunks folded dual-stream by TT_MINRED (PSUM + staged SBUF) with
    per-lane min accumulation. Host folds the two half-results.
  Phase B (d1, per (b,nchunk)): 4 banks [n=128, 512], an m-PAIR per row
    group (K=30, block-diagonal), two rounds per bank (cols 0:256/256:512).
    d1 = segmented tensor_reduce over s per bank.

Batch (B=16) is data-parallel across 8 NeuronCores (2 batches/core).
Host applies argsort/stick-breaking and superquadric area weighting.
"""

import sys

for _p in ("/opt/trn_rl_repo", "/root/.axon_site", "/root/.axon_site/_ro/trn_rl_repo",
           "/root/.axon_site/_ro/pypackages"):
    if _p not in sys.path:
        sys.path.append(_p)

import numpy as np

import concourse.bass as bass
import concourse.tile as tile
from concourse import bacc, mybir
from concourse.bass_utils import run_bass_kernel_spmd
from concourse import dve_ops as _dve_ops
from concourse.dve_ops import DveOp as _DveOp
from concourse.dve_spec import (
    Spec as _Spec, Src0 as _Src0, Src1 as _Src1, C0 as _C0, AluOp as _AluOp,
    minn as _minn, lower as _lower, _has_src1,
)
from concourse.dve_uop import DveOpSpec as _DveOpSpec


def _register_dve_op(name, spec):
    if name in _dve_ops._SUB_OPCODE_FOR_NAME:
        return next(op for op in _dve_ops.OPS if op.name == name)
    row = _dve_ops._CUSTOM_DVE_ROW_BASE + len(_dve_ops.OPS)
    assert row < 0x20
    _dve_ops._SUB_OPCODE_FOR_NAME[name] = row
    shas = {}
    for ver in ("v3", "v4"):
        tmp = _DveOpSpec(name=name, opcode=row, uops=_lower(spec, ver=ver),
                         rd1_en=_has_src1(spec))
        shas[ver] = tmp.sha(ver)
    op = _DveOp(name, spec, subdim=False, uops_sha=shas)
    _dve_ops.OPS.append(op)
    _dve_ops.CUSTOM_DVE_SPECS[name] = spec
    return op


TT_MINRED = _register_dve_op(
    "TT_MINRED_ANT",
    _Spec(
        body=_minn(_Src0, _Src1),
        accum=_AluOp.MIN,
        accum_init=_C0,
        reference=lambda in0, in1, s0, s1, imm2: np.minimum(
            in0.astype(np.float32), in1),
    ),
)

F32 = mybir.dt.float32
BF16 = mybir.dt.bfloat16
ALU = mybir.AluOpType

B, N, M, S = 16, 2048, 16, 128
CORES = 8
BPC = B // CORES
NCHUNK = N // 128
NQUAD = 8
KR = 15
FOUR_PI = 4.0 * np.pi

_PROGRAM = None
LAST_RESULTS = None


def _build_program():
    nc = bacc.Bacc("TRN2", target_bir_lowering=False, debug=False)

    a_stat_d = nc.dram_tensor("a_stat", [NQUAD, 4, KR, 128], BF16,
                              kind="ExternalInput").ap()
    a_mov_d = nc.dram_tensor("a_mov", [NQUAD, 4, KR, N], BF16,
                             kind="ExternalInput").ap()
    b_stat_d = nc.dram_tensor("b_stat", [BPC, NCHUNK, 2, 4, 30, 128], BF16,
                              kind="ExternalInput").ap()
    b_mov_d = nc.dram_tensor("b_mov", [BPC, 2, 4, 30, 256], BF16,
                             kind="ExternalInput").ap()
    d2o_d = nc.dram_tensor("d2o", [NQUAD, 128, 4, 2], F32, kind="ExternalOutput").ap()
    d1o_d = nc.dram_tensor("d1o", [BPC, NCHUNK, 128, M], F32,
                           kind="ExternalOutput").ap()

    from contextlib import ExitStack

    with tile.TileContext(nc) as tc, ExitStack() as ctx:
        const = ctx.enter_context(tc.tile_pool(name="const", bufs=1))
        p_ast = ctx.enter_context(tc.tile_pool(name="ast", bufs=2))
        p_amv = ctx.enter_context(tc.tile_pool(name="amv", bufs=2))
        p_bst = ctx.enter_context(tc.tile_pool(name="bst", bufs=4))
        p_hq = ctx.enter_context(tc.tile_pool(name="hq", bufs=2))
        p_stg = ctx.enter_context(tc.tile_pool(name="stg", bufs=2))
        p_scr = ctx.enter_context(tc.tile_pool(name="scr", bufs=2))
        p_d1x = ctx.enter_context(tc.tile_pool(name="d1x", bufs=4))

        # B moving operands: resident, tiny
        bmv = []
        for b in range(BPC):
            for r in range(2):
                t = const.tile([128, 256], BF16, tag=f"bmv{b}_{r}", name=f"bmv{b}_{r}")
                for j in range(4):
                    nc.sync.dma_start(out=t[32*j:32*j+30, :], in_=b_mov_d[b, r, j])
                bmv.append(t)

        # ---------------- phase A: d2 (min over n) ----------------
        psA_cm = tc.tile_pool(name="psA", bufs=2, space="PSUM")
        p_psA = psA_cm.__enter__()
        for q in range(NQUAD):
            ast = p_ast.tile([128, 128], BF16, tag="ast", name=f"ast{q}")
            amv = p_amv.tile([128, N], BF16, tag="amv", name=f"amv{q}")
            for j in range(4):
                nc.sync.dma_start(out=ast[32*j:32*j+KR, :], in_=a_stat_d[q, j])
                nc.sync.dma_start(out=amv[32*j:32*j+KR, :], in_=a_mov_d[q, j])
            hq = p_hq.tile([128, 4, 2], F32, tag="hq", name=f"hq{q}")
            stg = []
            for j in range(4):
                sj = p_stg.tile([128, 512], F32, tag=f"stg{j}", name=f"stg{q}_{j}")
                stg.append(sj)
            scr = p_scr.tile([128, 512], F32, tag="scr", name=f"scr{q}")
            for c in range(4):
                pA = []
                for j in range(4):
                    t = p_psA.tile([128, 512], F32, tag=f"pA{j}", name=f"pA{q}_{c}_{j}")
                    nc.tensor.matmul(
                        t[:], lhsT=ast[32*j:32*j+KR, :],
                        rhs=amv[32*j:32*j+KR, 512*c:512*(c+1)],
                        start=True, stop=True, tile_position=(32*j, 0))
                    pA.append(t)
                if c % 2 == 0:
                    for j in range(4):
                        nc.scalar.copy(stg[j][:], pA[j][:])
                else:
                    h = c // 2
                    for j in range(4):
                        nc.vector._custom_dve(
                            TT_MINRED, out=scr[:], in0=pA[j][:], in1=stg[j][:],
                            s0=3.0e38, accum_out=hq[:, j, h:h+1])
            nc.sync.dma_start(out=d2o_d[q], in_=hq[:])
        psA_cm.__exit__(None, None, None)

        # ---------------- phase B: d1 (min over s) ----------------
        psB_cm = tc.tile_pool(name="psB", bufs=2, space="PSUM")
        p_psB = psB_cm.__enter__()
        for i in range(32):
            bb, cc = i // 16, i % 16
            bst = p_bst.tile([128, 2, 128], BF16, tag="bst", name=f"bst{i}")
            for r in range(2):
                for j in range(4):
                    nc.sync.dma_start(out=bst[32*j:32*j+30, r, :],
                                      in_=b_stat_d[bb, cc, r, j])
            pB = []
            for j in range(4):
                t = p_psB.tile([128, 512], F32, tag=f"pB{j}", name=f"pB{i}_{j}")
                pB.append(t)
            for r in range(2):
                for j in range(4):
                    nc.tensor.matmul(
                        pB[j][:, 256*r:256*(r+1)],
                        lhsT=bst[32*j:32*j+30, r, :],
                        rhs=bmv[2*bb+r][32*j:32*j+30, :],
                        start=True, stop=True, tile_position=(32*j, 0))
            d1x = p_d1x.tile([128, M], F32, tag="d1x", name=f"d1x{i}")
            for j in range(4):
                nc.vector.tensor_reduce(
                    out=d1x[:, 4*j:4*j+4],
                    in_=pB[j][:].rearrange("p (m s) -> p m s", m=4),
                    axis=mybir.AxisListType.X, op=ALU.min)
            nc.sync.dma_start(out=d1o_d[bb, cc], in_=d1x[:])
        psB_cm.__exit__(None, None, None)

    nc.compile()
    return nc


def _get_program():
    global _PROGRAM
    if _PROGRAM is None:
        _PROGRAM = _build_program()
    return _PROGRAM


def _make_in_maps(pcl, prim):
    import ml_dtypes
    bf = ml_dtypes.bfloat16
    Xf = np.asarray(pcl, np.float32)      # (B, N, M, 3)
    Pf = np.asarray(prim, np.float32)     # (B, M, S, 3)
    Xhi = Xf.astype(bf).astype(np.float32)
    Xlo = (Xf - Xhi).astype(bf).astype(np.float32)
    Phi = Pf.astype(bf).astype(np.float32)
    Plo = (Pf - Phi).astype(bf).astype(np.float32)
    X64 = Xhi.astype(np.float64) + Xlo
    P64 = Phi.astype(np.float64) + Plo
    xx64 = np.einsum("bnmc,bnmc->bnm", X64, X64)
    pp64 = np.einsum("bmsc,bmsc->bms", P64, P64)

    def split3(v64):
        b0 = v64.astype(np.float32).astype(bf).astype(np.float64)
        r1 = v64 - b0
        b1 = r1.astype(np.float32).astype(bf).astype(np.float64)
        b2 = (r1 - b1).astype(np.float32).astype(bf).astype(np.float64)
        return np.stack([b0, b1, b2]).astype(np.float32)     # (3, ...)

    xx_b = split3(xx64)                   # (3, B, N, M)
    pp_b = split3(pp64)                   # (3, B, M, S)

    in_maps = []
    for core in range(CORES):
        bsl = slice(BPC * core, BPC * (core + 1))
        Xh, Xl = Xhi[bsl], Xlo[bsl]
        Ph, Pl = Phi[bsl], Plo[bsl]
        xxc = xx_b[:, bsl]                # (3, b, N, M)
        ppc = pp_b[:, bsl]                # (3, b, M, S)

        a_stat = np.empty((NQUAD, 4, KR, 128), np.float32)
        a_mov = np.empty((NQUAD, 4, KR, N), np.float32)
        for q in range(NQUAD):
            b, qq = q // 4, q % 4
            for j in range(4):
                m = 4 * qq + j
                PhT = Ph[b, m].T          # (3, S)
                PlT = Pl[b, m].T
                a_stat[q, j, 0:3] = -2.0 * PhT
                a_stat[q, j, 3:6] = -2.0 * PhT
                a_stat[q, j, 6:9] = -2.0 * PlT
                a_stat[q, j, 9:12] = ppc[:, b, m]
                a_stat[q, j, 12:15] = 1.0
                XhT = Xh[b, :, m, :].T    # (3, N)
                XlT = Xl[b, :, m, :].T
                a_mov[q, j, 0:3] = XhT
                a_mov[q, j, 3:6] = XlT
                a_mov[q, j, 6:9] = XhT
                a_mov[q, j, 9:12] = 1.0
                a_mov[q, j, 12:15] = xxc[:, b, :, m]

        b_stat = np.empty((BPC, NCHUNK, 2, 4, 30, 128), np.float32)
        b_mov = np.zeros((BPC, 2, 4, 30, 256), np.float32)
        for b in range(BPC):
            for r in range(2):
                for j in range(4):
                    for t in range(2):
                        m = 8 * r + 2 * j + t
                        r0, cs = 15 * t, slice(128 * t, 128 * (t + 1))
                        PhT = Ph[b, m].T
                        PlT = Pl[b, m].T
                        b_mov[b, r, j, r0+0:r0+3, cs] = PhT
                        b_mov[b, r, j, r0+3:r0+6, cs] = PlT
                        b_mov[b, r, j, r0+6:r0+9, cs] = PhT
                        b_mov[b, r, j, r0+9:r0+12, cs] = ppc[:, b, m]
                        b_mov[b, r, j, r0+12:r0+15, cs] = 1.0
                        for ccn in range(NCHUNK):
                            nsl = slice(128 * ccn, 128 * (ccn + 1))
                            XhT = Xh[b, nsl, m, :].T
                            XlT = Xl[b, nsl, m, :].T
                            b_stat[b, ccn, r, j, r0+0:r0+3] = -2.0 * XhT
                            b_stat[b, ccn, r, j, r0+3:r0+6] = -2.0 * XhT
                            b_stat[b, ccn, r, j, r0+6:r0+9] = -2.0 * XlT
                            b_stat[b, ccn, r, j, r0+9:r0+12] = 1.0
                            b_stat[b, ccn, r, j, r0+12:r0+15] = xxc[:, b, nsl, m]
        in_maps.append({
            "a_stat": a_stat.astype(bf), "a_mov": a_mov.astype(bf),
            "b_stat": b_stat.astype(bf), "b_mov": b_mov.astype(bf)})
    return in_maps


def kernel(pcl_transformed, primitive_points, size, probs, _trace=False):
    global LAST_RESULTS
    pcl = np.asarray(pcl_transformed, dtype=np.float32)
    prim = np.asarray(primitive_points, dtype=np.float32)
    size = np.asarray(size, dtype=np.float32)
    probs = np.asarray(probs, dtype=np.float32)

    nc = _get_program()
    in_maps = _make_in_maps(pcl, prim)
    res = run_bass_kernel_spmd(nc, in_maps, list(range(CORES)), trace=_trace)
    LAST_RESULTS = res

    d2min = np.empty((B, M, S), np.float64)
    d1 = np.empty((B, N, M), np.float64)
    for core in range(CORES):
        d2o = res.results[core]["d2o"].astype(np.float64)    # [8, 128, 4j, 2h]
        d2q = d2o.min(axis=3)                                # [8, 128(s), 4j]
        for q in range(NQUAD):
            b, qq = q // 4, q % 4
            for j in range(4):
                d2min[BPC * core + b, 4 * qq + j] = d2q[q, :, j]
        d1o = res.results[core]["d1o"].astype(np.float64)    # [2, 16, 128, 16]
        # col 4j+seg with seg=(2r+t) -> m = 8r + 2j + t
        perm = np.empty(M, np.int64)
        for j in range(4):
            for r in range(2):
                for t in range(2):
                    perm[8 * r + 2 * j + t] = 4 * j + 2 * r + t
        d1[BPC * core: BPC * (core + 1)] = (
            d1o[..., perm].reshape(BPC, N, M))

    p64v = probs.astype(np.float64)
    d1f = d1.reshape(B * N, M)
    order = np.argsort(d1f, axis=1, kind="stable")
    ps = np.take_along_axis(np.repeat(p64v, N, axis=0), order, axis=1)
    ncp = np.cumprod(1.0 - ps, axis=1)
    ncp = np.concatenate([np.ones((B * N, 1)), ncp[:, :-1]], axis=1)
    p2p_sum = float((np.take_along_axis(d1f, order, axis=1) * ps * ncp).sum())

    d2 = np.where(d2min >= 1e30, 0.0, d2min)

    s0 = size[..., 0].astype(np.float64)
    s1 = size[..., 1].astype(np.float64)
    s2 = size[..., 2].astype(np.float64)
    area = FOUR_PI * ((s0 * s1) ** 1.6 / 3 + (s0 * s2) ** 1.6 / 3
                      + (s1 * s2) ** 1.6 / 3) ** 0.625
    area = M * area / area.sum(axis=-1, keepdims=True)

    prim_to_pcl = float(
        (d2.mean(axis=-1) * probs.astype(np.float64) * area).sum() / (B * M))
    pcl_to_prim = float(p2p_sum / (B * N))

    total = np.float32(pcl_to_prim + prim_to_pcl)
    return (total,
            np.float32(pcl_to_prim),
            np.float32(prim_to_pcl),
            np.float32(0.0))


# revision 14
# speedup vs baseline: 2.0281x; 2.0281x over previous
"""Trainium2 Bass kernel for the DualLoss nn.Module (v3).

dist[b,m,s,n] = ||P[b,m,s] - X[b,n,m]||^2 via bf16 hi/lo-split matmuls
(K=15 contraction rows per m), with 4x PE row-tiling so four matmuls run
concurrently in the 128x128 array (tile_position=(32j,0)).

  Phase A (d2, per (b,mquad)): 4 banks [s=128, n-chunk 512], one m per
    row group (K=15). Even chunks staged PSUM->SBUF by the scalar engine;
    odd chunks folded dual-stream by TT_MINRED (PSUM + staged SBUF) with
    per-lane min accumulation. Host folds the two half-results.
  Phase B (d1, per (b,nchunk)): 4 banks [n=128, 512], an m-PAIR per row
    group (K=30, block-diagonal), two rounds per bank (cols 0:256/256:512).
    d1 = segmented tensor_reduce over s per bank.

Batch (B=16) is data-parallel across 8 NeuronCores (2 batches/core).
Host applies argsort/stick-breaking and superquadric area weighting.
"""

import sys

for _p in ("/opt/trn_rl_repo", "/root/.axon_site", "/root/.axon_site/_ro/trn_rl_repo",
           "/root/.axon_site/_ro/pypackages"):
    if _p not in sys.path:
        sys.path.append(_p)

import numpy as np

import concourse.bass as bass
import concourse.tile as tile
from concourse import bacc, mybir
from concourse.bass_utils import run_bass_kernel_spmd
from concourse import dve_ops as _dve_ops
from concourse.dve_ops import DveOp as _DveOp
from concourse.dve_spec import (
    Spec as _Spec, Src0 as _Src0, Src1 as _Src1, C0 as _C0, AluOp as _AluOp,
    minn as _minn, lower as _lower, _has_src1,
)
from concourse.dve_uop import DveOpSpec as _DveOpSpec


def _register_dve_op(name, spec):
    if name in _dve_ops._SUB_OPCODE_FOR_NAME:
        return next(op for op in _dve_ops.OPS if op.name == name)
    row = _dve_ops._CUSTOM_DVE_ROW_BASE + len(_dve_ops.OPS)
    assert row < 0x20
    _dve_ops._SUB_OPCODE_FOR_NAME[name] = row
    shas = {}
    for ver in ("v3", "v4"):
        tmp = _DveOpSpec(name=name, opcode=row, uops=_lower(spec, ver=ver),
                         rd1_en=_has_src1(spec))
        shas[ver] = tmp.sha(ver)
    op = _DveOp(name, spec, subdim=False, uops_sha=shas)
    _dve_ops.OPS.append(op)
    _dve_ops.CUSTOM_DVE_SPECS[name] = spec
    return op


TT_MINRED = _register_dve_op(
    "TT_MINRED_ANT",
    _Spec(
        body=_minn(_Src0, _Src1),
        accum=_AluOp.MIN,
        accum_init=_C0,
        reference=lambda in0, in1, s0, s1, imm2: np.minimum(
            in0.astype(np.float32), in1),
    ),
)

F32 = mybir.dt.float32
BF16 = mybir.dt.bfloat16
ALU = mybir.AluOpType

B, N, M, S = 16, 2048, 16, 128
CORES = 8
BPC = B // CORES
NCHUNK = N // 128
NQUAD = 8
KR = 15
FOUR_PI = 4.0 * np.pi

_PROGRAM = None
LAST_RESULTS = None


def _build_program():
    nc = bacc.Bacc("TRN2", target_bir_lowering=False, debug=False)

    a_stat_d = nc.dram_tensor("a_stat", [4, KR, NQUAD, 128], BF16,
                              kind="ExternalInput").ap()
    a_mov_d = nc.dram_tensor("a_mov", [NQUAD, 4, KR, N], BF16,
                             kind="ExternalInput").ap()
    b_stat_d = nc.dram_tensor("b_stat", [4, 30, BPC, NCHUNK, 2, 128], BF16,
                              kind="ExternalInput").ap()
    b_mov_d = nc.dram_tensor("b_mov", [BPC, 2, 4, 30, 256], BF16,
                             kind="ExternalInput").ap()
    d2o_d = nc.dram_tensor("d2o", [NQUAD, 128, 4, 2], F32, kind="ExternalOutput").ap()
    d1o_d = nc.dram_tensor("d1o", [BPC, NCHUNK, 128, M], F32,
                           kind="ExternalOutput").ap()

    from contextlib import ExitStack

    with tile.TileContext(nc) as tc, ExitStack() as ctx:
        const = ctx.enter_context(tc.tile_pool(name="const", bufs=1))
        p_ast = ctx.enter_context(tc.tile_pool(name="ast", bufs=2))
        p_amv = ctx.enter_context(tc.tile_pool(name="amv", bufs=2))
        p_bst = ctx.enter_context(tc.tile_pool(name="bst", bufs=4))
        p_hq = ctx.enter_context(tc.tile_pool(name="hq", bufs=2))
        p_stg = ctx.enter_context(tc.tile_pool(name="stg", bufs=2))
        p_scr = ctx.enter_context(tc.tile_pool(name="scr", bufs=2))
        p_d1x = ctx.enter_context(tc.tile_pool(name="d1x", bufs=4))

        # resident stationaries: A (2KB/part) + B (16KB/part)
        astall = const.tile([128, NQUAD, 128], BF16, tag="astall")
        for j in range(4):
            nc.sync.dma_start(out=astall[32*j:32*j+KR], in_=a_stat_d[j])
        bstall = const.tile([128, BPC, NCHUNK, 2, 128], BF16, tag="bstall")
        for j in range(4):
            nc.sync.dma_start(out=bstall[32*j:32*j+30], in_=b_stat_d[j])

        # B moving operands: resident, tiny
        bmv = []
        for b in range(BPC):
            for r in range(2):
                t = const.tile([128, 256], BF16, tag=f"bmv{b}_{r}", name=f"bmv{b}_{r}")
                for j in range(4):
                    nc.sync.dma_start(out=t[32*j:32*j+30, :], in_=b_mov_d[b, r, j])
                bmv.append(t)

        # ---------------- phase A: d2 (min over n) ----------------
        psA_cm = tc.tile_pool(name="psA", bufs=2, space="PSUM")
        p_psA = psA_cm.__enter__()
        for q in range(NQUAD):
            amv = p_amv.tile([128, N], BF16, tag="amv", name=f"amv{q}")
            for j in range(4):
                nc.sync.dma_start(out=amv[32*j:32*j+KR, :], in_=a_mov_d[q, j])
            hq = p_hq.tile([128, 4, 2], F32, tag="hq", name=f"hq{q}")
            stg = []
            for j in range(4):
                sj = p_stg.tile([128, 512], F32, tag=f"stg{j}", name=f"stg{q}_{j}")
                stg.append(sj)
            scr = p_scr.tile([128, 512], F32, tag="scr", name=f"scr{q}")
            for c in range(4):
                pA = []
                for j in range(4):
                    t = p_psA.tile([128, 512], F32, tag=f"pA{j}", name=f"pA{q}_{c}_{j}")
                    nc.tensor.matmul(
                        t[:], lhsT=astall[32*j:32*j+KR, q, :],
                        rhs=amv[32*j:32*j+KR, 512*c:512*(c+1)],
                        start=True, stop=True, tile_position=(32*j, 0))
                    pA.append(t)
                if c % 2 == 0:
                    for j in range(4):
                        nc.scalar.copy(stg[j][:], pA[j][:])
                else:
                    h = c // 2
                    for j in range(4):
                        nc.vector._custom_dve(
                            TT_MINRED, out=scr[:], in0=pA[j][:], in1=stg[j][:],
                            s0=3.0e38, accum_out=hq[:, j, h:h+1])
            nc.sync.dma_start(out=d2o_d[q], in_=hq[:])
        psA_cm.__exit__(None, None, None)

        # ---------------- phase B: d1 (min over s) ----------------
        psB_cm = tc.tile_pool(name="psB", bufs=2, space="PSUM")
        p_psB = psB_cm.__enter__()
        for i in range(32):
            bb, cc = i // 16, i % 16
            pB = []
            for j in range(4):
                t = p_psB.tile([128, 512], F32, tag=f"pB{j}", name=f"pB{i}_{j}")
                pB.append(t)
            for r in range(2):
                for j in range(4):
                    nc.tensor.matmul(
                        pB[j][:, 256*r:256*(r+1)],
                        lhsT=bstall[32*j:32*j+30, bb, cc, r, :],
                        rhs=bmv[2*bb+r][32*j:32*j+30, :],
                        start=True, stop=True, tile_position=(32*j, 0))
            d1x = p_d1x.tile([128, M], F32, tag="d1x", name=f"d1x{i}")
            for j in range(4):
                nc.vector.tensor_reduce(
                    out=d1x[:, 4*j:4*j+4],
                    in_=pB[j][:].rearrange("p (m s) -> p m s", m=4),
                    axis=mybir.AxisListType.X, op=ALU.min)
            nc.sync.dma_start(out=d1o_d[bb, cc], in_=d1x[:])
        psB_cm.__exit__(None, None, None)

    nc.compile()
    return nc


def _get_program():
    global _PROGRAM
    if _PROGRAM is None:
        _PROGRAM = _build_program()
    return _PROGRAM


def _make_in_maps(pcl, prim):
    import ml_dtypes
    bf = ml_dtypes.bfloat16
    Xf = np.asarray(pcl, np.float32)      # (B, N, M, 3)
    Pf = np.asarray(prim, np.float32)     # (B, M, S, 3)
    Xhi = Xf.astype(bf).astype(np.float32)
    Xlo = (Xf - Xhi).astype(bf).astype(np.float32)
    Phi = Pf.astype(bf).astype(np.float32)
    Plo = (Pf - Phi).astype(bf).astype(np.float32)
    X64 = Xhi.astype(np.float64) + Xlo
    P64 = Phi.astype(np.float64) + Plo
    xx64 = np.einsum("bnmc,bnmc->bnm", X64, X64)
    pp64 = np.einsum("bmsc,bmsc->bms", P64, P64)

    def split3(v64):
        b0 = v64.astype(np.float32).astype(bf).astype(np.float64)
        r1 = v64 - b0
        b1 = r1.astype(np.float32).astype(bf).astype(np.float64)
        b2 = (r1 - b1).astype(np.float32).astype(bf).astype(np.float64)
        return np.stack([b0, b1, b2]).astype(np.float32)     # (3, ...)

    xx_b = split3(xx64)                   # (3, B, N, M)
    pp_b = split3(pp64)                   # (3, B, M, S)

    in_maps = []
    for core in range(CORES):
        bsl = slice(BPC * core, BPC * (core + 1))
        Xh, Xl = Xhi[bsl], Xlo[bsl]
        Ph, Pl = Phi[bsl], Plo[bsl]
        xxc = xx_b[:, bsl]                # (3, b, N, M)
        ppc = pp_b[:, bsl]                # (3, b, M, S)

        a_stat = np.empty((4, KR, NQUAD, 128), np.float32)
        a_mov = np.empty((NQUAD, 4, KR, N), np.float32)
        for q in range(NQUAD):
            b, qq = q // 4, q % 4
            for j in range(4):
                m = 4 * qq + j
                PhT = Ph[b, m].T          # (3, S)
                PlT = Pl[b, m].T
                a_stat[j, 0:3, q] = -2.0 * PhT
                a_stat[j, 3:6, q] = -2.0 * PhT
                a_stat[j, 6:9, q] = -2.0 * PlT
                a_stat[j, 9:12, q] = ppc[:, b, m]
                a_stat[j, 12:15, q] = 1.0
                XhT = Xh[b, :, m, :].T    # (3, N)
                XlT = Xl[b, :, m, :].T
                a_mov[q, j, 0:3] = XhT
                a_mov[q, j, 3:6] = XlT
                a_mov[q, j, 6:9] = XhT
                a_mov[q, j, 9:12] = 1.0
                a_mov[q, j, 12:15] = xxc[:, b, :, m]

        b_stat = np.empty((4, 30, BPC, NCHUNK, 2, 128), np.float32)
        b_mov = np.zeros((BPC, 2, 4, 30, 256), np.float32)
        for b in range(BPC):
            for r in range(2):
                for j in range(4):
                    for t in range(2):
                        m = 8 * r + 2 * j + t
                        r0, cs = 15 * t, slice(128 * t, 128 * (t + 1))
                        PhT = Ph[b, m].T
                        PlT = Pl[b, m].T
                        b_mov[b, r, j, r0+0:r0+3, cs] = PhT
                        b_mov[b, r, j, r0+3:r0+6, cs] = PlT
                        b_mov[b, r, j, r0+6:r0+9, cs] = PhT
                        b_mov[b, r, j, r0+9:r0+12, cs] = ppc[:, b, m]
                        b_mov[b, r, j, r0+12:r0+15, cs] = 1.0
                        for ccn in range(NCHUNK):
                            nsl = slice(128 * ccn, 128 * (ccn + 1))
                            XhT = Xh[b, nsl, m, :].T
                            XlT = Xl[b, nsl, m, :].T
                            b_stat[j, r0+0:r0+3, b, ccn, r] = -2.0 * XhT
                            b_stat[j, r0+3:r0+6, b, ccn, r] = -2.0 * XhT
                            b_stat[j, r0+6:r0+9, b, ccn, r] = -2.0 * XlT
                            b_stat[j, r0+9:r0+12, b, ccn, r] = 1.0
                            b_stat[j, r0+12:r0+15, b, ccn, r] = xxc[:, b, nsl, m]
        in_maps.append({
            "a_stat": a_stat.astype(bf), "a_mov": a_mov.astype(bf),
            "b_stat": b_stat.astype(bf), "b_mov": b_mov.astype(bf)})
    return in_maps


def kernel(pcl_transformed, primitive_points, size, probs, _trace=False):
    global LAST_RESULTS
    pcl = np.asarray(pcl_transformed, dtype=np.float32)
    prim = np.asarray(primitive_points, dtype=np.float32)
    size = np.asarray(size, dtype=np.float32)
    probs = np.asarray(probs, dtype=np.float32)

    nc = _get_program()
    in_maps = _make_in_maps(pcl, prim)
    res = run_bass_kernel_spmd(nc, in_maps, list(range(CORES)), trace=_trace)
    LAST_RESULTS = res

    d2min = np.empty((B, M, S), np.float64)
    d1 = np.empty((B, N, M), np.float64)
    for core in range(CORES):
        d2o = res.results[core]["d2o"].astype(np.float64)    # [8, 128, 4j, 2h]
        d2q = d2o.min(axis=3)                                # [8, 128(s), 4j]
        for q in range(NQUAD):
            b, qq = q // 4, q % 4
            for j in range(4):
                d2min[BPC * core + b, 4 * qq + j] = d2q[q, :, j]
        d1o = res.results[core]["d1o"].astype(np.float64)    # [2, 16, 128, 16]
        # col 4j+seg with seg=(2r+t) -> m = 8r + 2j + t
        perm = np.empty(M, np.int64)
        for j in range(4):
            for r in range(2):
                for t in range(2):
                    perm[8 * r + 2 * j + t] = 4 * j + 2 * r + t
        d1[BPC * core: BPC * (core + 1)] = (
            d1o[..., perm].reshape(BPC, N, M))

    p64v = probs.astype(np.float64)
    d1f = d1.reshape(B * N, M)
    order = np.argsort(d1f, axis=1, kind="stable")
    ps = np.take_along_axis(np.repeat(p64v, N, axis=0), order, axis=1)
    ncp = np.cumprod(1.0 - ps, axis=1)
    ncp = np.concatenate([np.ones((B * N, 1)), ncp[:, :-1]], axis=1)
    p2p_sum = float((np.take_along_axis(d1f, order, axis=1) * ps * ncp).sum())

    d2 = np.where(d2min >= 1e30, 0.0, d2min)

    s0 = size[..., 0].astype(np.float64)
    s1 = size[..., 1].astype(np.float64)
    s2 = size[..., 2].astype(np.float64)
    area = FOUR_PI * ((s0 * s1) ** 1.6 / 3 + (s0 * s2) ** 1.6 / 3
                      + (s1 * s2) ** 1.6 / 3) ** 0.625
    area = M * area / area.sum(axis=-1, keepdims=True)

    prim_to_pcl = float(
        (d2.mean(axis=-1) * probs.astype(np.float64) * area).sum() / (B * M))
    pcl_to_prim = float(p2p_sum / (B * N))

    total = np.float32(pcl_to_prim + prim_to_pcl)
    return (total,
            np.float32(pcl_to_prim),
            np.float32(prim_to_pcl),
            np.float32(0.0))


# revision 15
# speedup vs baseline: 2.1473x; 1.0588x over previous
"""Trainium2 Bass kernel for the DualLoss nn.Module (v3).

dist[b,m,s,n] = ||P[b,m,s] - X[b,n,m]||^2 via bf16 hi/lo-split matmuls
(K=15 contraction rows per m), with 4x PE row-tiling so four matmuls run
concurrently in the 128x128 array (tile_position=(32j,0)).

  Phase A (d2, per (b,mquad)): 4 banks [s=128, n-chunk 512], one m per
    row group (K=15). Even chunks staged PSUM->SBUF by the scalar engine;
    odd chunks folded dual-stream by TT_MINRED (PSUM + staged SBUF) with
    per-lane min accumulation. Host folds the two half-results.
  Phase B (d1, per (b,nchunk)): 4 banks [n=128, 512], an m-PAIR per row
    group (K=30, block-diagonal), two rounds per bank (cols 0:256/256:512).
    d1 = segmented tensor_reduce over s per bank.

Batch (B=16) is data-parallel across 8 NeuronCores (2 batches/core).
Host applies argsort/stick-breaking and superquadric area weighting.
"""

import sys

for _p in ("/opt/trn_rl_repo", "/root/.axon_site", "/root/.axon_site/_ro/trn_rl_repo",
           "/root/.axon_site/_ro/pypackages"):
    if _p not in sys.path:
        sys.path.append(_p)

import numpy as np

import concourse.bass as bass
import concourse.tile as tile
from concourse import bacc, mybir
from concourse.bass_utils import run_bass_kernel_spmd
from concourse import dve_ops as _dve_ops
from concourse.dve_ops import DveOp as _DveOp
from concourse.dve_spec import (
    Spec as _Spec, Src0 as _Src0, Src1 as _Src1, C0 as _C0, AluOp as _AluOp,
    minn as _minn, lower as _lower, _has_src1,
)
from concourse.dve_uop import DveOpSpec as _DveOpSpec


def _register_dve_op(name, spec):
    if name in _dve_ops._SUB_OPCODE_FOR_NAME:
        return next(op for op in _dve_ops.OPS if op.name == name)
    row = _dve_ops._CUSTOM_DVE_ROW_BASE + len(_dve_ops.OPS)
    assert row < 0x20
    _dve_ops._SUB_OPCODE_FOR_NAME[name] = row
    shas = {}
    for ver in ("v3", "v4"):
        tmp = _DveOpSpec(name=name, opcode=row, uops=_lower(spec, ver=ver),
                         rd1_en=_has_src1(spec))
        shas[ver] = tmp.sha(ver)
    op = _DveOp(name, spec, subdim=False, uops_sha=shas)
    _dve_ops.OPS.append(op)
    _dve_ops.CUSTOM_DVE_SPECS[name] = spec
    return op


TT_MINRED = _register_dve_op(
    "TT_MINRED_ANT",
    _Spec(
        body=_minn(_Src0, _Src1),
        accum=_AluOp.MIN,
        accum_init=_C0,
        reference=lambda in0, in1, s0, s1, imm2: np.minimum(
            in0.astype(np.float32), in1),
    ),
)

F32 = mybir.dt.float32
BF16 = mybir.dt.bfloat16
ALU = mybir.AluOpType

B, N, M, S = 16, 2048, 16, 128
CORES = 8
BPC = B // CORES
NCHUNK = N // 128
NQUAD = 8
KR = 15
FOUR_PI = 4.0 * np.pi

_PROGRAM = None
LAST_RESULTS = None


def _build_program():
    nc = bacc.Bacc("TRN2", target_bir_lowering=False, debug=False)

    a_stat_d = nc.dram_tensor("a_stat", [4, KR, NQUAD, 128], BF16,
                              kind="ExternalInput").ap()
    a_mov_d = nc.dram_tensor("a_mov", [NQUAD, 4, KR, N], BF16,
                             kind="ExternalInput").ap()
    b_stat_d = nc.dram_tensor("b_stat", [4, 30, BPC, NCHUNK, 2, 128], BF16,
                              kind="ExternalInput").ap()
    b_mov_d = nc.dram_tensor("b_mov", [4, 30, BPC, 2, 256], BF16,
                             kind="ExternalInput").ap()
    d2o_d = nc.dram_tensor("d2o", [NQUAD, 128, 4, 2], F32, kind="ExternalOutput").ap()
    d1o_d = nc.dram_tensor("d1o", [BPC, NCHUNK, 128, M], F32,
                           kind="ExternalOutput").ap()

    from contextlib import ExitStack

    with tile.TileContext(nc) as tc, ExitStack() as ctx:
        const = ctx.enter_context(tc.tile_pool(name="const", bufs=1))
        p_ast = ctx.enter_context(tc.tile_pool(name="ast", bufs=2))
        p_amv = ctx.enter_context(tc.tile_pool(name="amv", bufs=2))
        p_bst = ctx.enter_context(tc.tile_pool(name="bst", bufs=4))
        p_hq = ctx.enter_context(tc.tile_pool(name="hq", bufs=2))
        p_stg = ctx.enter_context(tc.tile_pool(name="stg", bufs=2))
        p_scr = ctx.enter_context(tc.tile_pool(name="scr", bufs=2))
        p_d1x = ctx.enter_context(tc.tile_pool(name="d1x", bufs=4))

        # resident stationaries: A loads now; B tiles declared, loaded
        # after quad 0 is issued (so phase A starts immediately)
        astall = const.tile([128, NQUAD, 128], BF16, tag="astall")
        for j in range(4):
            nc.sync.dma_start(out=astall[32*j:32*j+KR], in_=a_stat_d[j])
        bstall = const.tile([128, BPC, NCHUNK, 2, 128], BF16, tag="bstall")
        bmvall = const.tile([128, BPC, 2, 256], BF16, tag="bmvall")

        # ---------------- phase A: d2 (min over n) ----------------
        psA_cm = tc.tile_pool(name="psA", bufs=2, space="PSUM")
        p_psA = psA_cm.__enter__()
        for q in range(NQUAD):
            amv = p_amv.tile([128, N], BF16, tag="amv", name=f"amv{q}")
            for j in range(4):
                nc.sync.dma_start(out=amv[32*j:32*j+KR, :], in_=a_mov_d[q, j])
            if q == 1:
                for j in range(4):
                    nc.sync.dma_start(out=bstall[32*j:32*j+30], in_=b_stat_d[j])
                    nc.sync.dma_start(out=bmvall[32*j:32*j+30], in_=b_mov_d[j])
            hq = p_hq.tile([128, 4, 2], F32, tag="hq", name=f"hq{q}")
            stg = []
            for j in range(4):
                sj = p_stg.tile([128, 512], F32, tag=f"stg{j}", name=f"stg{q}_{j}")
                stg.append(sj)
            scr = p_scr.tile([128, 512], F32, tag="scr", name=f"scr{q}")
            for c in range(4):
                pA = []
                for j in range(4):
                    t = p_psA.tile([128, 512], F32, tag=f"pA{j}", name=f"pA{q}_{c}_{j}")
                    nc.tensor.matmul(
                        t[:], lhsT=astall[32*j:32*j+KR, q, :],
                        rhs=amv[32*j:32*j+KR, 512*c:512*(c+1)],
                        start=True, stop=True, tile_position=(32*j, 0))
                    pA.append(t)
                if c % 2 == 0:
                    for j in range(4):
                        nc.scalar.copy(stg[j][:], pA[j][:])
                else:
                    h = c // 2
                    for j in range(4):
                        nc.vector._custom_dve(
                            TT_MINRED, out=scr[:], in0=pA[j][:], in1=stg[j][:],
                            s0=3.0e38, accum_out=hq[:, j, h:h+1])
            nc.sync.dma_start(out=d2o_d[q], in_=hq[:])
        psA_cm.__exit__(None, None, None)

        # ---------------- phase B: d1 (min over s) ----------------
        psB_cm = tc.tile_pool(name="psB", bufs=2, space="PSUM")
        p_psB = psB_cm.__enter__()
        for i in range(32):
            bb, cc = i // 16, i % 16
            pB = []
            for j in range(4):
                t = p_psB.tile([128, 512], F32, tag=f"pB{j}", name=f"pB{i}_{j}")
                pB.append(t)
            for r in range(2):
                for j in range(4):
                    nc.tensor.matmul(
                        pB[j][:, 256*r:256*(r+1)],
                        lhsT=bstall[32*j:32*j+30, bb, cc, r, :],
                        rhs=bmvall[32*j:32*j+30, bb, r, :],
                        start=True, stop=True, tile_position=(32*j, 0))
            d1x = p_d1x.tile([128, M], F32, tag="d1x", name=f"d1x{i}")
            for j in range(4):
                nc.vector.tensor_reduce(
                    out=d1x[:, 4*j:4*j+4],
                    in_=pB[j][:].rearrange("p (m s) -> p m s", m=4),
                    axis=mybir.AxisListType.X, op=ALU.min)
            nc.sync.dma_start(out=d1o_d[bb, cc], in_=d1x[:])
        psB_cm.__exit__(None, None, None)

    nc.compile()
    return nc


def _get_program():
    global _PROGRAM
    if _PROGRAM is None:
        _PROGRAM = _build_program()
    return _PROGRAM


def _make_in_maps(pcl, prim):
    import ml_dtypes
    bf = ml_dtypes.bfloat16
    Xf = np.asarray(pcl, np.float32)      # (B, N, M, 3)
    Pf = np.asarray(prim, np.float32)     # (B, M, S, 3)
    Xhi = Xf.astype(bf).astype(np.float32)
    Xlo = (Xf - Xhi).astype(bf).astype(np.float32)
    Phi = Pf.astype(bf).astype(np.float32)
    Plo = (Pf - Phi).astype(bf).astype(np.float32)
    X64 = Xhi.astype(np.float64) + Xlo
    P64 = Phi.astype(np.float64) + Plo
    xx64 = np.einsum("bnmc,bnmc->bnm", X64, X64)
    pp64 = np.einsum("bmsc,bmsc->bms", P64, P64)

    def split3(v64):
        b0 = v64.astype(np.float32).astype(bf).astype(np.float64)
        r1 = v64 - b0
        b1 = r1.astype(np.float32).astype(bf).astype(np.float64)
        b2 = (r1 - b1).astype(np.float32).astype(bf).astype(np.float64)
        return np.stack([b0, b1, b2]).astype(np.float32)     # (3, ...)

    xx_b = split3(xx64)                   # (3, B, N, M)
    pp_b = split3(pp64)                   # (3, B, M, S)

    in_maps = []
    for core in range(CORES):
        bsl = slice(BPC * core, BPC * (core + 1))
        Xh, Xl = Xhi[bsl], Xlo[bsl]
        Ph, Pl = Phi[bsl], Plo[bsl]
        xxc = xx_b[:, bsl]                # (3, b, N, M)
        ppc = pp_b[:, bsl]                # (3, b, M, S)

        a_stat = np.empty((4, KR, NQUAD, 128), np.float32)
        a_mov = np.empty((NQUAD, 4, KR, N), np.float32)
        for q in range(NQUAD):
            b, qq = q // 4, q % 4
            for j in range(4):
                m = 4 * qq + j
                PhT = Ph[b, m].T          # (3, S)
                PlT = Pl[b, m].T
                a_stat[j, 0:3, q] = -2.0 * PhT
                a_stat[j, 3:6, q] = -2.0 * PhT
                a_stat[j, 6:9, q] = -2.0 * PlT
                a_stat[j, 9:12, q] = ppc[:, b, m]
                a_stat[j, 12:15, q] = 1.0
                XhT = Xh[b, :, m, :].T    # (3, N)
                XlT = Xl[b, :, m, :].T
                a_mov[q, j, 0:3] = XhT
                a_mov[q, j, 3:6] = XlT
                a_mov[q, j, 6:9] = XhT
                a_mov[q, j, 9:12] = 1.0
                a_mov[q, j, 12:15] = xxc[:, b, :, m]

        b_stat = np.empty((4, 30, BPC, NCHUNK, 2, 128), np.float32)
        b_mov = np.zeros((4, 30, BPC, 2, 256), np.float32)
        for b in range(BPC):
            for r in range(2):
                for j in range(4):
                    for t in range(2):
                        m = 8 * r + 2 * j + t
                        r0, cs = 15 * t, slice(128 * t, 128 * (t + 1))
                        PhT = Ph[b, m].T
                        PlT = Pl[b, m].T
                        b_mov[j, r0+0:r0+3, b, r, cs] = PhT
                        b_mov[j, r0+3:r0+6, b, r, cs] = PlT
                        b_mov[j, r0+6:r0+9, b, r, cs] = PhT
                        b_mov[j, r0+9:r0+12, b, r, cs] = ppc[:, b, m]
                        b_mov[j, r0+12:r0+15, b, r, cs] = 1.0
                        for ccn in range(NCHUNK):
                            nsl = slice(128 * ccn, 128 * (ccn + 1))
                            XhT = Xh[b, nsl, m, :].T
                            XlT = Xl[b, nsl, m, :].T
                            b_stat[j, r0+0:r0+3, b, ccn, r] = -2.0 * XhT
                            b_stat[j, r0+3:r0+6, b, ccn, r] = -2.0 * XhT
                            b_stat[j, r0+6:r0+9, b, ccn, r] = -2.0 * XlT
                            b_stat[j, r0+9:r0+12, b, ccn, r] = 1.0
                            b_stat[j, r0+12:r0+15, b, ccn, r] = xxc[:, b, nsl, m]
        in_maps.append({
            "a_stat": a_stat.astype(bf), "a_mov": a_mov.astype(bf),
            "b_stat": b_stat.astype(bf), "b_mov": b_mov.astype(bf)})
    return in_maps


def kernel(pcl_transformed, primitive_points, size, probs, _trace=False):
    global LAST_RESULTS
    pcl = np.asarray(pcl_transformed, dtype=np.float32)
    prim = np.asarray(primitive_points, dtype=np.float32)
    size = np.asarray(size, dtype=np.float32)
    probs = np.asarray(probs, dtype=np.float32)

    nc = _get_program()
    in_maps = _make_in_maps(pcl, prim)
    res = run_bass_kernel_spmd(nc, in_maps, list(range(CORES)), trace=_trace)
    LAST_RESULTS = res

    d2min = np.empty((B, M, S), np.float64)
    d1 = np.empty((B, N, M), np.float64)
    for core in range(CORES):
        d2o = res.results[core]["d2o"].astype(np.float64)    # [8, 128, 4j, 2h]
        d2q = d2o.min(axis=3)                                # [8, 128(s), 4j]
        for q in range(NQUAD):
            b, qq = q // 4, q % 4
            for j in range(4):
                d2min[BPC * core + b, 4 * qq + j] = d2q[q, :, j]
        d1o = res.results[core]["d1o"].astype(np.float64)    # [2, 16, 128, 16]
        # col 4j+seg with seg=(2r+t) -> m = 8r + 2j + t
        perm = np.empty(M, np.int64)
        for j in range(4):
            for r in range(2):
                for t in range(2):
                    perm[8 * r + 2 * j + t] = 4 * j + 2 * r + t
        d1[BPC * core: BPC * (core + 1)] = (
            d1o[..., perm].reshape(BPC, N, M))

    p64v = probs.astype(np.float64)
    d1f = d1.reshape(B * N, M)
    order = np.argsort(d1f, axis=1, kind="stable")
    ps = np.take_along_axis(np.repeat(p64v, N, axis=0), order, axis=1)
    ncp = np.cumprod(1.0 - ps, axis=1)
    ncp = np.concatenate([np.ones((B * N, 1)), ncp[:, :-1]], axis=1)
    p2p_sum = float((np.take_along_axis(d1f, order, axis=1) * ps * ncp).sum())

    d2 = np.where(d2min >= 1e30, 0.0, d2min)

    s0 = size[..., 0].astype(np.float64)
    s1 = size[..., 1].astype(np.float64)
    s2 = size[..., 2].astype(np.float64)
    area = FOUR_PI * ((s0 * s1) ** 1.6 / 3 + (s0 * s2) ** 1.6 / 3
                      + (s1 * s2) ** 1.6 / 3) ** 0.625
    area = M * area / area.sum(axis=-1, keepdims=True)

    prim_to_pcl = float(
        (d2.mean(axis=-1) * probs.astype(np.float64) * area).sum() / (B * M))
    pcl_to_prim = float(p2p_sum / (B * N))

    total = np.float32(pcl_to_prim + prim_to_pcl)
    return (total,
            np.float32(pcl_to_prim),
            np.float32(prim_to_pcl),
            np.float32(0.0))
